# revision 55
# baseline (speedup 1.0000x reference)
"""3-layer GCN (GCNConv + residual + relu, global add pool, MLP softmax) on 8
Trainium2 NeuronCores.

Sharding: nodes/edges partitioned by destination across the 8 cores, with a
host-side LPT rebalance (node relabeling) that equalizes per-(window, chunk)
edge counts across cores to minimize SPMD slot padding. Per layer: each core
computes its shard of the message table xw' = (Wg.T h) * dinv_src, AllGathers
it (bf16, in 4 overlapping quarter chunks, staggered so each quarter's
AllGather overlaps the previous layer's gathers; tables double-buffered by
layer parity), then gathers per-edge rows with dma_gather and segment-sums
them into per-64-dst-window PSUM accumulators via one-hot matmuls (sel built
on-device comparing fp16 dstrel against an iota row). The host balancer
levels every (window, chunk) cell to exactly 256 slots, so tiles never span
windows: one 64-wide MM per gathered tile. Self-loop terms are computed
locally in the epilogue (never gathered); the next layer's xw'=h@Wg chunks
and the global-add-pool partials are emitted inside the epilogues for
cross-phase overlap. All cores run the IDENTICAL program; per-core variation
lives entirely in data (gather indices, sel values, padding). Pooled
[64,128] partials are AllReduced; the tiny classifier is replicated.
"""
import os
import numpy as np
import ml_dtypes

import concourse.bacc as bacc
import concourse.bass as bass
import concourse.mybir as mybir
import concourse.tile as tile
from concourse.bass_utils import run_bass_kernel_spmd

NCORES = 8
G = 64     # graphs in batch
C = 2      # classes
W = 64     # dst window width (one-hot / psum granularity)
SBWIN = 16  # dst windows per superblock (16*64 = 1024 dst = 2 psum banks)
SLAB = 32  # sel tiles built per is_equal op
GPART = 32  # tiles per dma_gather (gb recycle granularity)

NPAD = 12544          # padded nodes per core
NWIN = NPAD // W      # 196 dst windows per core
NW = NPAD // 128      # 98 "a-tiles" (128-node blocks) per core
NSB = -(-NWIN // SBWIN)  # 13 superblocks
# gather-table quarters: CYCLIC overlapping a-tile ranges (32 tiles each,
# starts spaced ~24.5) so every quarter overlaps both neighbours. Edges whose
# source a-tile lies in an overlap may use either chunk, giving the host
# balancer freedom to level per-(window, chunk) counts to a 256-slot cap
# (= exactly 2 gather tiles, so cell boundaries stay tile-aligned).
QS = [0, 25, 49, 74]       # cyclic start tile of each quarter
NWQ = 30                   # a-tiles per quarter
CHR = NCORES * 128 * NWQ   # 30720 rows per chunk table
CAPT = 256                 # target slot cap per (window, chunk) cell


def _qtiles(q):
    """a-tile list of quarter q, in srow order."""
    return [(QS[q] + i) % NW for i in range(NWQ)]

bf16 = ml_dtypes.bfloat16
_cache = {}


def _ceil(a, b):
    return -(-a // b)


# --------------------------------------------------------------------------
# host preprocessing
# --------------------------------------------------------------------------
def _balance_nodes(indeg):
    """LPT: assign nodes to NCORES*NWIN global windows of W slots, balancing
    the gathered in-degree sum per window. Returns (core, loc) per node."""
    import heapq
    N = len(indeg)
    GW = NCORES * NWIN
    assert GW * W >= N
    order = np.argsort(-indeg, kind="stable")
    heap = [(0, w) for w in range(GW)]
    heapq.heapify(heap)
    room = np.full(GW, W, np.int64)
    fill = np.zeros(GW, np.int64)
    gwin = np.empty(N, np.int64)
    pos = np.empty(N, np.int64)
    for nd in order:
        while True:
            load, wbin = heapq.heappop(heap)
            if room[wbin] > 0:
                break
        gwin[nd] = wbin
        pos[nd] = fill[wbin]
        fill[wbin] += 1
        room[wbin] -= 1
        if room[wbin] > 0:
            heapq.heappush(heap, (load + int(indeg[nd]), wbin))
    core = gwin % NCORES
    windex = gwin // NCORES
    loc = windex * W + pos
    return core, loc


def _preprocess(x, edge_index, batch):
    N, D = x.shape
    assert D == 128

    src = np.asarray(edge_index[0], np.int64)
    dst = np.asarray(edge_index[1], np.int64)
    E = len(src)
    deg = np.bincount(dst, minlength=N).astype(np.float64) + 1.0
    dinv = (deg ** -0.5).astype(np.float32)
    indeg = np.bincount(dst, minlength=N)  # gathered (non-loop) in-degree

    core_of, loc_of = _balance_nodes(indeg)

    # --- per-core edge slot layout ------------------------------------
    csrc, lsrc = core_of[src], loc_of[src]
    cdst, ldst = core_of[dst], loc_of[dst]
    p_s = lsrc % 128
    a_s = lsrc // 128
    w64 = ldst // W

    # chunk assignment: a-tile t belongs to quarter q iff (t-QS[q]) mod 98
    # < 32. Forced when only one quarter covers it; flexible (overlap of
    # quarters p and p+1 mod 4) edges are split per (core, dst-window) to
    # level the four chunk counts to <= CAPT.
    memb = np.stack([(a_s - QS[q]) % NW < NWQ for q in range(4)])  # [4, E]
    nmemb = memb.sum(axis=0)
    assert ((nmemb == 1) | (nmemb == 2)).all()
    ch = np.full(E, -1, np.int64)
    flex = np.full(E, -1, np.int64)   # pool id p: chunk p or (p+1)%4
    for q in range(4):
        only = memb[q] & (nmemb == 1)
        ch[only] = q
        both = memb[q] & memb[(q + 1) % 4]
        flex[both] = q
    assert ((ch >= 0) ^ (flex >= 0)).all()

    # per (core, window, chunk) forced counts and flexible pool counts
    fcnt = np.zeros((NCORES, NWIN, 4), np.int64)
    fmask = ch >= 0
    np.add.at(fcnt, (cdst[fmask], w64[fmask], ch[fmask]), 1)
    xcnt = np.zeros((NCORES, NWIN, 4), np.int64)
    xmask = flex >= 0
    np.add.at(xcnt, (cdst[xmask], w64[xmask], flex[xmask]), 1)

    # cyclic balance: take_left[p] = # of pool-p edges assigned to chunk p
    # (rest go to chunk (p+1)%4). Equalizing sweeps, then overflow-draining
    # passes pushing any chunk above CAPT down through its pools.
    take_left = xcnt.copy()  # start: all pool edges to left chunk
    ccnt = fcnt.copy()
    for p in range(4):
        ccnt[:, :, p] += take_left[:, :, p]
    for it in range(24):
        for p in range(4):
            a = ccnt[:, :, p]
            b = ccnt[:, :, (p + 1) % 4]
            if it < 8:
                shift = (a - b) // 2  # >0: move left->right
            else:
                # drain overflow only
                shift = (np.maximum(a - CAPT, 0)
                         - np.maximum(b - CAPT, 0))
            shift = np.clip(shift, take_left[:, :, p] - xcnt[:, :, p],
                            take_left[:, :, p])
            take_left[:, :, p] -= shift
            ccnt[:, :, p] -= shift
            ccnt[:, :, (p + 1) % 4] += shift

    # resolve flexible edges: first take_left (by order) -> chunk p, rest p+1
    if xmask.any():
        xi = np.flatnonzero(xmask)
        okey = cdst[xi] * (NWIN * 4) + w64[xi] * 4 + flex[xi]
        oorder = np.argsort(okey, kind="stable")
        xi = xi[oorder]
        okey = okey[oorder]
        ost = np.r_[0, np.flatnonzero(np.diff(okey)) + 1]
        og = np.zeros(len(okey), np.int64)
        og[ost[1:]] = 1
        og = np.cumsum(og)
        opos = np.arange(len(okey)) - ost[og]
        tl = take_left.reshape(-1)[okey]
        ch[xi] = np.where(opos < tl, flex[xi], (flex[xi] + 1) % 4)
    assert (ch >= 0).all()

    qs_arr = np.asarray(QS, np.int64)
    srow = (csrc * (128 * NWQ) + p_s * NWQ + (a_s - qs_arr[ch]) % NW)
    assert srow.min() >= 0 and srow.max() < 32768

    cell = w64 * 4 + ch
    key = cdst * (NWIN * 4) + cell
    counts = np.bincount(key, minlength=NCORES * NWIN * 4)
    counts = counts.reshape(NCORES, NWIN * 4)
    cap = _ceil(np.maximum(counts.max(axis=0), 1), 128) * 128
    novf = int((cap > CAPT).sum())
    if novf > 40:
        import sys
        print(f"[kernel] warning: {novf} cells over CAPT", file=sys.stderr)

    # stream layout: sb -> ch -> w; groups padded to x128
    cell_slot_off = np.zeros(NWIN * 4, np.int64)
    groups = []           # (sb, ch, tile_off, n_tiles)
    slot_w_list = []      # per-slot window id, -1 = pad
    so = 0
    for sb in range(NSB):
        ws = range(sb * SBWIN, min((sb + 1) * SBWIN, NWIN))
        for c4 in range(4):
            g_so = so
            for w_ in ws:
                cid = w_ * 4 + c4
                cell_slot_off[cid] = so
                slot_w_list.append(np.full(int(cap[cid]), w_, np.int64))
                so += int(cap[cid])
            pad = (-(so - g_so)) % 128
            if pad:
                slot_w_list.append(np.full(pad, -1, np.int64))
                so += pad
            groups.append((sb, c4, g_so // 128, (so - g_so) // 128))
    SLOTS = so
    NT = SLOTS // 128
    slot_w = np.concatenate(slot_w_list)

    # static per-tile structure: crossing-free -> exactly one MM per tile
    first_w = np.zeros(NT, np.int64)
    mm_by_group = {}      # (sb,c4) -> [(t, rel)]
    for (sb, c4, g_off, g_nt) in groups:
        mms = []
        for ti in range(g_nt):
            t = g_off + ti
            ws_here = slot_w[t * 128:(t + 1) * 128]
            ws_u = np.unique(ws_here[ws_here >= 0])
            if len(ws_u) == 0:
                continue
            assert len(ws_u) == 1, "tile spans >1 window (caps not x128?)"
            w0 = int(ws_u[0])
            first_w[t] = w0
            mms.append((t, w0 - sb * SBWIN))
        mm_by_group[(sb, c4)] = mms

    # psum accumulation flags per (sb, bank): first/last in emission order
    mm_flags = {}         # (sb,c4) -> [(t, rel, start, stop)]
    for sb in range(NSB):
        seq = []
        for c4 in range(4):
            for m in mm_by_group[(sb, c4)]:
                seq.append((c4, m))
        firstmm = {}
        lastmm = {}
        for i, (c4, (t, rel)) in enumerate(seq):
            bk = rel // 8
            if bk not in firstmm:
                firstmm[bk] = i
            lastmm[bk] = i
        nwin_sb = min(SBWIN, NWIN - sb * SBWIN)
        for bk in range(_ceil(nwin_sb, 8)):
            assert bk in firstmm, f"psum bank {sb}/{bk} has no MM"
        for c4 in range(4):
            mm_flags[(sb, c4)] = []
        for i, (c4, (t, rel)) in enumerate(seq):
            bk = rel // 8
            mm_flags[(sb, c4)].append(
                (t, rel, i == firstmm[bk], i == lastmm[bk]))

    # --- per-core slot placement --------------------------------------
    order = np.lexsort((srow, cell, cdst))
    core_s = cdst[order]
    cell_s = cell[order]
    keyall = core_s * (NWIN * 4) + cell_s
    starts = np.r_[0, np.flatnonzero(np.diff(keyall)) + 1]
    gid = np.zeros(len(keyall), np.int64)
    gid[starts[1:]] = 1
    gid = np.cumsum(gid)
    pos = np.arange(len(keyall)) - starts[gid]
    slot = cell_slot_off[cell_s] + pos
    assert (pos < cap[cell_s]).all()

    gidx_all = np.zeros((NCORES, SLOTS), np.int16)
    dstrel_all = np.full((NCORES, SLOTS), -1.0, np.float32)
    gidx_all[core_s, slot] = srow[order].astype(np.int16)
    tile_of_slot = slot // 128
    dstrel_all[core_s, slot] = (ldst[order] - first_w[tile_of_slot] * W
                                ).astype(np.float32)
    dr = dstrel_all[core_s, slot]
    assert (dr >= 0).all() and (dr < W).all()

    # --- device layouts -----------------------------------------------
    gidx_dev = np.tile(
        gidx_all.reshape(NCORES, SLOTS // 16, 16).transpose(0, 2, 1), (1, 8, 1)
    ).copy()                                           # [8, 128, SLOTS//16]
    dstrel_dev = dstrel_all.reshape(NCORES, NT, 128).transpose(0, 2, 1) \
        .astype(np.float16).copy()                     # [8, 128, NT]

    batch = np.asarray(batch, np.int64)
    brel = np.full((NCORES, NPAD), -1.0, np.float32)
    brel[core_of, loc_of] = batch.astype(np.float32)
    batchrel_dev = brel.reshape(NCORES, NW, 128).transpose(0, 2, 1).copy()

    x = np.asarray(x, np.float32)
    xt_dev = np.zeros((NCORES, 128, NPAD), bf16)
    xt_dev[core_of, :, loc_of] = x.astype(bf16)
    dinvT_dev = np.zeros((NCORES, 128, NPAD), bf16)
    dv = np.zeros((NCORES, NPAD), np.float32)
    dv[core_of, loc_of] = dinv
    for cc in range(NCORES):
        dinvT_dev[cc] = np.broadcast_to(dv[cc], (128, NPAD)).astype(bf16)

    meta = dict(N=N, NT=NT, SLOTS=SLOTS, groups=groups, mm_flags=mm_flags)
    data = dict(gidx=gidx_dev, dstrel=dstrel_dev, batchrel=batchrel_dev,
                xt=xt_dev, dinvt=dinvT_dev)
    return meta, data


# --------------------------------------------------------------------------
# device program
# --------------------------------------------------------------------------
def _build(meta, L, ablate=()):
    ablate = set(ablate)
    f32 = mybir.dt.float32
    f16 = mybir.dt.float16
    b16 = mybir.dt.bfloat16
    i16 = mybir.dt.int16
    NT, SLOTS = meta["NT"], meta["SLOTS"]
    groups, mm_flags = meta["groups"], meta["mm_flags"]
    rg = [list(range(NCORES))]
    # NOTE: is_equal is NOT supported on the Pool engine by the real ISA
    # (walrus rejects it) -- sel slabs must stay on DVE.
    pool_every = int(os.environ.get("SEL_POOL_EVERY", "0"))
    epi_pool = os.environ.get("EPI_POOL", "0") == "1"

    nc = bacc.Bacc("TRN2", target_bir_lowering=False, debug=False,
                   num_devices=NCORES)
    d_xt = nc.dram_tensor("xt", [128, NPAD], b16, kind="ExternalInput")
    d_dinvt = nc.dram_tensor("dinvt", [128, NPAD], b16, kind="ExternalInput")
    d_gidx = nc.dram_tensor("gidx", [128, SLOTS // 16], i16, kind="ExternalInput")
    d_dstrel = nc.dram_tensor("dstrel", [128, NT], f16, kind="ExternalInput")
    d_batchrel = nc.dram_tensor("batchrel", [128, NW], f32, kind="ExternalInput")
    d_w0 = nc.dram_tensor("w0", [128, 128], b16, kind="ExternalInput")
    d_wg = nc.dram_tensor("wg", [L, 128, 128], b16, kind="ExternalInput")
    d_wc1 = nc.dram_tensor("wc1", [128, 128], b16, kind="ExternalInput")
    d_wc2 = nc.dram_tensor("wc2", [128, C], b16, kind="ExternalInput")
    d_b0 = nc.dram_tensor("b0", [128, 1], f32, kind="ExternalInput")
    d_bg = nc.dram_tensor("bg", [L, 128, 1], f32, kind="ExternalInput")
    d_bc1 = nc.dram_tensor("bc1", [128, 1], f32, kind="ExternalInput")
    d_bc2m = nc.dram_tensor("bc2m", [G, C], f32, kind="ExternalInput")
    d_iota16 = nc.dram_tensor("iota16", [128, 128], f16, kind="ExternalInput")
    d_iotag = nc.dram_tensor("iotag", [128, G], f32, kind="ExternalInput")
    d_id128 = nc.dram_tensor("id128", [128, 128], b16, kind="ExternalInput")
    d_idg = nc.dram_tensor("idg", [G, G], b16, kind="ExternalInput")
    d_out = nc.dram_tensor("out", [G, C], f32, kind="ExternalOutput")

    ag_in = [nc.dram_tensor(f"ag_in{q}", [128, NWQ, 128], b16)
             for q in range(4)]
    # double-buffered by layer parity so next layer's AllGather overlaps
    # this layer's gathers instead of WAR-serializing on the table
    xw_q = [[nc.dram_tensor(f"xw_q{b}_{q}", [CHR, 128], b16,
                            addr_space="Shared")
             for q in range(4)] for b in range(2)]
    pool_in = nc.dram_tensor("pool_in", [G, 128], f32)
    pool_out = nc.dram_tensor("pool_out", [G, 128], f32, addr_space="Shared")

    Relu = mybir.ActivationFunctionType.Relu
    Exp = mybir.ActivationFunctionType.Exp
    Copy = mybir.ActivationFunctionType.Copy
    AT = mybir.AluOpType

    with tile.TileContext(nc) as tc:
        with (
            tc.tile_pool(name="state", bufs=1) as state,
            tc.tile_pool(name="wpool", bufs=1) as wpool,
            tc.tile_pool(name="xin", bufs=3) as xinp,
            tc.tile_pool(name="gbf", bufs=4) as gbfp,
            tc.tile_pool(name="sel", bufs=5) as selp,
            tc.tile_pool(name="epi", bufs=4) as epip,
            tc.tile_pool(name="cls", bufs=2) as clsp,
            tc.tile_pool(name="psxw", bufs=2, space="PSUM") as psxw,
            tc.tile_pool(name="pstr", bufs=2, space="PSUM") as pstr,
            tc.tile_pool(name="pswin", bufs=2, space="PSUM") as pswin,
        ):
            # ---- persistent state + constants ----
            h = state.tile([128, NPAD], b16, tag="h")
            xws = state.tile([128, NPAD], b16, tag="xws")
            xwp = state.tile([128, NW, 128], b16, tag="xwp")
            dinvT = state.tile([128, NPAD], b16, tag="dinvT")
            dstrel = state.tile([128, NT], f16, tag="dstrel")
            gidxS = state.tile([128, SLOTS // 16], i16, tag="gidxS")
            nc.sync.dma_start(dinvT[:], d_dinvt[:])
            nc.sync.dma_start(dstrel[:], d_dstrel[:])
            nc.sync.dma_start(gidxS[:], d_gidx[:])

            w0 = wpool.tile([128, 128], b16, tag="w0")
            nc.sync.dma_start(w0[:], d_w0[:])
            wg = wpool.tile([128, L, 128], b16, tag="wg")
            nc.sync.dma_start(wg[:], d_wg.rearrange("l p f -> p l f"))
            wc1 = wpool.tile([128, 128], b16, tag="wc1")
            nc.sync.dma_start(wc1[:], d_wc1[:])
            wc2 = wpool.tile([128, C], b16, tag="wc2")
            nc.sync.dma_start(wc2[:], d_wc2[:])
            b0 = wpool.tile([128, 1], f32, tag="b0")
            nc.sync.dma_start(b0[:], d_b0[:])
            bg = wpool.tile([128, L], f32, tag="bg")
            nc.sync.dma_start(bg[:], d_bg.rearrange("l p o -> p (l o)"))
            bc1 = wpool.tile([128, 1], f32, tag="bc1")
            nc.sync.dma_start(bc1[:], d_bc1[:])
            bc2m = wpool.tile([G, C], f32, tag="bc2m")
            nc.sync.dma_start(bc2m[:], d_bc2m[:])
            iota16 = wpool.tile([128, 128], f16, tag="iota16")
            nc.sync.dma_start(iota16[:], d_iota16[:])
            iotag = wpool.tile([128, G], f32, tag="iotag")
            nc.sync.dma_start(iotag[:], d_iotag[:])
            id128 = wpool.tile([128, 128], b16, tag="id128")
            nc.sync.dma_start(id128[:], d_id128[:])
            idg = wpool.tile([G, G], b16, tag="idg")
            nc.sync.dma_start(idg[:], d_idg[:])
            batchrel = wpool.tile([128, NW], f32, tag="batchrel")
            nc.sync.dma_start(batchrel[:], d_batchrel[:])

            nchunks = _ceil(NPAD, 512)
            sel_ctr = [0]
            pool_state = {}
            # quarter q's a-tiles are all transposed once the epilogue of
            # this superblock (resp. stage-1 chunk) has run
            _AGQ_AT_SB = {3: [0], 6: [1], 10: [2], 12: [3]}
            _AGQ_AT_K = {7: [0], 13: [1], 20: [2], 24: [3]}

            def emit_phase_a_chunk(l, k):
                """xws/xwp for h cols [512k, ...) using layer-l weights."""
                c0 = k * 512
                cw = min(512, NPAD - c0)
                ps = psxw.tile([128, cw], f32, tag="psxw", name="psA")
                nc.tensor.matmul(ps[:], lhsT=wg[:, l, :],
                                 rhs=h[:, c0:c0 + cw], start=True, stop=True)
                nc.vector.tensor_tensor(out=xws[:, c0:c0 + cw], in0=ps[:],
                                        in1=dinvT[:, c0:c0 + cw], op=AT.mult)
                for j in range(cw // 128):
                    a = (c0 + j * 128) // 128
                    pst = pstr.tile([128, 128], b16, tag="pstr", name="pstA")
                    nc.tensor.transpose(
                        pst[:], xws[:, c0 + j * 128:c0 + (j + 1) * 128],
                        id128[:])
                    nc.scalar.copy(out=xwp[:, a, :], in_=pst[:])

            def emit_ag_parts(q, buf):
                """thunk list: ag_in staging, then the collective (or its
                modeled per-rank copies, spread so they don't monopolize
                the DMA queue in one block)."""
                def stage():
                    s_ = QS[q]
                    n1 = min(NWQ, NW - s_)
                    nc.sync.dma_start(ag_in[q][:, 0:n1, :],
                                      xwp[:, s_:s_ + n1, :])
                    if n1 < NWQ:
                        nc.sync.dma_start(ag_in[q][:, n1:NWQ, :],
                                          xwp[:, 0:NWQ - n1, :])
                parts = [stage]
                if "noag" in ablate:
                    sz = 128 * NWQ * 128
                    def copy(r):
                        nc.sync.dma_start(
                            bass.AP(xw_q[buf][q], r * sz, ag_in[q][:].ap),
                            ag_in[q][:])
                    for r in range(NCORES):
                        parts.append(lambda r=r: copy(r))
                else:
                    # give the staging DMA two gather-group times to land
                    # before the collective occupies the in-order Pool SEQ
                    # (it would block later gather descriptor-gen while
                    # waiting on its input)
                    parts.append(lambda: None)
                    parts.append(lambda: None)
                    parts.append(lambda: nc.gpsimd.collective_compute(
                        "AllGather", AT.bypass, ins=[ag_in[q][:]],
                        outs=[xw_q[buf][q][:]], replica_groups=rg))
                return parts

            def emit_pool_sb(sb):
                """accumulate global-add-pool partials for sb's a-tiles."""
                if "psp" not in pool_state:
                    pool_state["psp"] = psxw.tile([G, 128], f32, tag="psxw",
                                                  name="psp")
                psp = pool_state["psp"]
                a_lo = sb * (SBWIN * W // 128)
                a_hi = min(a_lo + SBWIN * W // 128, NW)
                an = a_hi - a_lo
                bsel = selp.tile([128, an, G], b16, tag="sel", name="bsel")
                in0 = bass.AP(batchrel.tensor, batchrel[:, a_lo:a_hi].offset,
                              [batchrel[:].ap[0], [1, an], [0, G]])
                in1 = bass.AP(iotag.tensor, iotag[:].offset,
                              [iotag[:].ap[0], [0, an], [1, G]])
                nc.vector.tensor_tensor(out=bsel[:], in0=in0, in1=in1,
                                        op=AT.is_equal)
                for a in range(a_lo, a_hi):
                    pst = pstr.tile([128, 128], b16, tag="pstr", name="pstP")
                    nc.tensor.transpose(pst[:], h[:, a * 128:(a + 1) * 128],
                                        id128[:])
                    hn = clsp.tile([128, 128], b16, tag="hn", name="hn")
                    nc.scalar.copy(out=hn[:], in_=pst[:])
                    nc.tensor.matmul(psp[:], lhsT=bsel[:, a - a_lo, :],
                                     rhs=hn[:],
                                     start=(a == 0), stop=(a == NW - 1))

            def emit_phase_b(l):
                """gather + one-hot matmul segment-sum + epilogue, per sb.

                Cross-phase work (next layer's phase A, AllGathers, pooling)
                is queued and drained one item per gather group so the DVE/PE
                queues never see a burst that starves the sel->MM pipeline."""
                ps_sb = {}
                pending = []
                for (sb, c4, g_off, g_nt) in groups:
                    if sb not in ps_sb:
                        ps_sb.clear()
                        ps_sb[sb] = pswin.tile([128, SBWIN * W], f32,
                                               name="pswin_t", tag="pswin")
                    ps = ps_sb[sb]
                    if g_nt > 0:
                        mms = mm_flags[(sb, c4)]
                        mm_by_t = {}
                        for m in mms:
                            mm_by_t.setdefault(m[0], []).append(m)
                        # split each group into GPART-tile gathers so gb
                        # buffers recycle at finer granularity
                        for p0 in range(0, g_nt, GPART):
                            pn = min(GPART, g_nt - p0)
                            slots = pn * 128
                            gb = gbfp.tile([128, pn, 128], b16, tag="gbf",
                                           name="gb")
                            so = (g_off + p0) * 128
                            nc.gpsimd.dma_gather(
                                gb[:], xw_q[l % 2][c4][:],
                                gidxS[:, so // 16:(so + slots) // 16],
                                slots, slots, 128, single_packet=False)
                            t0 = g_off + p0
                            eng = (nc.gpsimd if pool_every > 0 and
                                   sel_ctr[0] % pool_every == pool_every - 1
                                   else nc.vector)
                            sel_ctr[0] += 1
                            st = selp.tile([128, pn, W], b16, tag="sel",
                                           name="st")
                            in0 = bass.AP(
                                dstrel.tensor, dstrel[:, t0:t0 + pn].offset,
                                [dstrel[:].ap[0], [1, pn], [0, W]])
                            in1 = bass.AP(
                                iota16.tensor, iota16[:].offset,
                                [iota16[:].ap[0], [0, pn], [1, W]])
                            eng.tensor_tensor(out=st[:], in0=in0, in1=in1,
                                              op=AT.is_equal)
                            for si in range(pn):
                                t = t0 + si
                                for (t_, rel, st_f, sp_f) in \
                                        mm_by_t.get(t, []):
                                    nc.tensor.matmul(
                                        ps[:, rel * W:(rel + 1) * W],
                                        lhsT=gb[:, si, :],
                                        rhs=st[:, si, :],
                                        start=bool(st_f), stop=bool(sp_f))
                    if pending:
                        pending.pop(0)()
                    if c4 == 3:
                        # epilogue: h[:,sb] = relu(h + dinv*(ps + xws) + bg)
                        nwin_sb = min(SBWIN, NWIN - sb * SBWIN)
                        cs = slice(sb * SBWIN * W, sb * SBWIN * W + nwin_sb * W)
                        u = epip.tile([128, SBWIN * W], f32, tag="u", name="u")
                        un = nwin_sb * W
                        nc.vector.tensor_tensor(out=u[:, :un],
                                                in0=ps[:, :un],
                                                in1=xws[:, cs], op=AT.add)
                        nc.vector.tensor_tensor(out=u[:, :un], in0=u[:, :un],
                                                in1=dinvT[:, cs], op=AT.mult)
                        # Pool engine can't touch PSUM; this add is all-SBUF
                        eng0 = nc.gpsimd if epi_pool else nc.vector
                        eng0.tensor_tensor(out=u[:, :un], in0=u[:, :un],
                                           in1=h[:, cs], op=AT.add)
                        nc.scalar.activation(out=h[:, cs], in_=u[:, :un],
                                             func=Relu, bias=bg[:, l:l + 1])
                        # h[:, sb] is final for this layer: queue the next
                        # layer's phase A for these columns (or the pooling
                        # partials after the last layer), plus any AllGather
                        # quarter whose a-tiles just completed.
                        if l < L - 1:
                            for k in range(2 * sb,
                                           min(2 * sb + 2, nchunks)):
                                pending.append(
                                    lambda k=k: emit_phase_a_chunk(l + 1, k))
                            for q in _AGQ_AT_SB.get(sb, []):
                                pending.extend(
                                    emit_ag_parts(q, (l + 1) % 2))
                        else:
                            pending.append(lambda sb=sb: emit_pool_sb(sb))
                while pending:
                    pending.pop(0)()

            import os as _os
            for _krep in range(int(_os.environ.get("BENCH_KREP", "1"))):
                # ---- stage 1 (h = relu(W0.T xT + b0)) fused with layer-0
                # phase A ----
                pool_state.clear()
                for k in range(nchunks):
                    c0 = k * 512
                    cw = min(512, NPAD - c0)
                    xts = xinp.tile([128, cw], b16, tag="xts", name="xts")
                    nc.sync.dma_start(xts[:], d_xt[:, c0:c0 + cw])
                    ps = psxw.tile([128, cw], f32, tag="psxw", name="ps1")
                    nc.tensor.matmul(ps[:], lhsT=w0[:], rhs=xts[:],
                                     start=True, stop=True)
                    nc.scalar.activation(out=h[:, c0:c0 + cw], in_=ps[:],
                                         func=Relu, bias=b0[:])
                    emit_phase_a_chunk(0, k)
                    for q in _AGQ_AT_K.get(k, []):
                        for part in emit_ag_parts(q, 0):
                            part()

                # ---- GCN layers (phase A of l+1 and pooling are emitted
                # inside the epilogues) ----
                for l in range(L):
                    emit_phase_b(l)

                # ---- global add pool: AllReduce partials ----
                pool_sb = clsp.tile([G, 128], f32, tag="poolsb", name="poolsb")
                nc.vector.tensor_copy(out=pool_sb[:], in_=pool_state["psp"][:])
                nc.sync.dma_start(pool_in[:], pool_sb[:])
                if "noar" in ablate:
                    nc.sync.dma_start(
                        bass.AP(pool_out, 0, pool_in[:].ap), pool_in[:])
                else:
                    nc.gpsimd.collective_compute(
                        "AllReduce", AT.add, ins=[pool_in[:]],
                        outs=[pool_out[:]], replica_groups=rg)

                # ---- classifier (replicated) ----
                pooled_f = clsp.tile([G, 128], f32, tag="pooledf",
                                     name="pooledf")
                nc.sync.dma_start(pooled_f[:], pool_out[:])
                pooled_b = clsp.tile([G, 128], b16, tag="pooledb",
                                     name="pooledb")
                nc.vector.tensor_copy(out=pooled_b[:], in_=pooled_f[:])
                pstp = pstr.tile([128, G], b16, tag="pstr", name="pstC")
                nc.tensor.transpose(pstp[:], pooled_b[:], idg[:])
                pooledT = clsp.tile([128, G], b16, tag="pooledT",
                                    name="pooledT")
                nc.vector.tensor_copy(out=pooledT[:], in_=pstp[:])
                psz = pstr.tile([128, G], f32, tag="pstr", name="psz")
                nc.tensor.matmul(psz[:], lhsT=wc1[:], rhs=pooledT[:],
                                 start=True, stop=True)
                zt = clsp.tile([128, G], b16, tag="zt", name="zt")
                nc.scalar.activation(out=zt[:], in_=psz[:], func=Relu,
                                     bias=bc1[:])
                pslg = pstr.tile([G, C], f32, tag="pstr", name="pslg")
                nc.tensor.matmul(pslg[:], lhsT=zt[:], rhs=wc2[:],
                                 start=True, stop=True)
                lg = clsp.tile([G, C], f32, tag="lg", name="lg")
                nc.vector.tensor_tensor(out=lg[:], in0=pslg[:], in1=bc2m[:],
                                        op=AT.add)
                # softmax over C (free dim)
                mx = clsp.tile([G, 1], f32, tag="mx", name="mx")
                nc.vector.tensor_reduce(out=mx[:], in_=lg[:],
                                        axis=mybir.AxisListType.X, op=AT.max)
                nmx = clsp.tile([G, 1], f32, tag="nmx", name="nmx")
                nc.vector.tensor_scalar_mul(nmx[:], mx[:], -1.0)
                ex = clsp.tile([G, C], f32, tag="ex", name="ex")
                nc.scalar.activation(out=ex[:], in_=lg[:], func=Exp,
                                     bias=nmx[:])
                sm = clsp.tile([G, 1], f32, tag="sm", name="sm")
                nc.vector.tensor_reduce(out=sm[:], in_=ex[:],
                                        axis=mybir.AxisListType.X, op=AT.add)
                rs = clsp.tile([G, 1], f32, tag="rs", name="rs")
                nc.vector.reciprocal(rs[:], sm[:])
                prob = clsp.tile([G, C], f32, tag="prob", name="prob")
                nc.vector.tensor_scalar_mul(prob[:], ex[:], rs[:])
                nc.sync.dma_start(d_out[:], prob[:])

    nc.compile()
    return nc


# --------------------------------------------------------------------------
# entry point
# --------------------------------------------------------------------------
def kernel(x, edge_index, batch, W0, b0, Wg, bg, Wc1, bc1, Wc2, bc2,
           **extra):
    x = np.asarray(x, np.float32)
    edge_index = np.asarray(edge_index)
    batch = np.asarray(batch)
    W0 = np.asarray(W0, np.float32)
    Wg = np.asarray(Wg, np.float32)
    L = Wg.shape[0]

    key = (x.shape, edge_index.shape,
           hash(edge_index.tobytes()), hash(np.asarray(batch).tobytes()))
    if key not in _cache:
        meta, data = _preprocess(x, edge_index, batch)
        nc = _build(meta, L)
        _cache.clear()
        _cache[key] = (meta, data, nc)
    meta, data, nc = _cache[key]

    iota16 = np.broadcast_to(np.arange(128, dtype=np.float16),
                             (128, 128)).copy()
    iotag = np.broadcast_to(np.arange(G, dtype=np.float32), (128, G)).copy()
    common = dict(
        w0=W0.astype(bf16).view(np.uint16),
        wg=Wg.astype(bf16).view(np.uint16),
        wc1=np.asarray(Wc1, np.float32).astype(bf16).view(np.uint16),
        wc2=np.asarray(Wc2, np.float32).astype(bf16).view(np.uint16),
        b0=np.asarray(b0, np.float32).reshape(128, 1),
        bg=np.asarray(bg, np.float32).reshape(L, 128, 1),
        bc1=np.asarray(bc1, np.float32).reshape(128, 1),
        bc2m=np.broadcast_to(np.asarray(bc2, np.float32), (G, C)).copy(),
        iota16=iota16.view(np.uint16), iotag=iotag,
        id128=np.eye(128, dtype=np.float32).astype(bf16).view(np.uint16),
        idg=np.eye(G, dtype=np.float32).astype(bf16).view(np.uint16),
    )
    in_maps = []
    for c in range(NCORES):
        m = dict(common)
        m["xt"] = data["xt"][c].view(np.uint16)
        m["dinvt"] = data["dinvt"][c].view(np.uint16)
        m["gidx"] = data["gidx"][c]
        m["dstrel"] = data["dstrel"][c].view(np.uint16)
        m["batchrel"] = data["batchrel"][c]
        in_maps.append(m)

    trace = os.environ.get("BASS_KERNEL_TRACE", "0") == "1"
    res = run_bass_kernel_spmd(nc, in_maps, list(range(NCORES)), trace=trace)
    kernel._last_exec_ns = res.exec_time_ns
    kernel._last_results = res
    return np.asarray(res.results[0]["out"], np.float32)


kernel._last_exec_ns = None
